# revision 15
# baseline (speedup 1.0000x reference)
"""MoE LoRA linear layer kernel for Trainium2, data-parallel over 8 NeuronCores.

Math (per token n):
    down = h @ down_w.T                      [N, 64]
    mask[n, r] = val[n, k] if idx[n, k] == r else 0   (indices distinct per row)
    out = (down * mask) @ up_w.T             [N, 4096]

Sharding: tokens split 8 ways (2048/core); LoRA weights replicated.

Key layout decisions (all host-side prep; HW does both matmuls + masking):
  - h is transposed + bf16-cast on the host and packed so each token tile
    is one fully contiguous 4 MiB DMA ([128, 32*512] per tile).  This
    removes the 512 on-device PE transposes + 8.4M elements of PSUM->SBUF
    copies the natural-layout path needs.
  - The top-k scatter (idx/val -> dense [64, NT] maskT) is host-packed like
    the baseline's idx/val chunk repack; the value multiply happens on
    device (DVE) against the down-projection PSUM result.
  - Output is stored bf16 (2e-2 rel-err budget; bf16 ~4e-3) and upcast on
    the host, halving store traffic.

Per-core pipeline (4 token tiles of 512):
  1. one 4 MiB contiguous DMA loads hT tile [128, 32*512] bf16
  2. 32 accumulating bf16 matmuls -> psum_dn [64, 512]
  3. resT = psum_dn * maskT slice (DVE, bf16 out)
  4. per 128-token chunk: 8 matmuls [K=64, N=512] -> psum, DVE/ACT copy
     (bf16 downcast) into out_sb [128, 4096], one 1 MiB store per chunk

HBM traffic per core: 16 MiB in + 16 MiB out + ~1.3 MiB weights ~= 33 MiB,
vs ~50 MiB PE/DVE-bound work in the f32 natural-layout baseline.
"""

import sys

for p in ("/opt/trn_rl_repo", "/opt/pypackages"):
    if p not in sys.path:
        sys.path.insert(0, p)

import numpy as np

N, D_IN, D_OUT, RANK, TOPK = 16384, 4096, 4096, 64, 8
NCORES = 8
NT = N // NCORES          # tokens per core = 2048
P = 128                   # partitions
TT = 512                  # token tile (down-matmul free dim, one PSUM bank)
NKC = D_IN // P           # 32 contraction chunks for down proj
NTILES = NT // TT         # 4 token tiles per core
NJ = TT // P              # 4 x 128-token chunks per tile
OT = 512                  # output col tile (one PSUM bank)
NOT = D_OUT // OT         # 8 output col tiles

_CACHE = {}


def _build_program():
    import concourse.bacc as bacc
    import concourse.mybir as mybir
    from concourse import tile

    f32 = mybir.dt.float32
    bf16 = mybir.dt.bfloat16
    # Bacc (not plain Bass): its finalize() runs move_matmul_waits_to_-
    # ldweights + generate_event_semaphores, which split semaphore waits to
    # satisfy the TRN2 one-wait-per-instruction constraint.
    nc = bacc.Bacc()

    ht = nc.declare_dram_parameter("ht", [NTILES * P, NKC * TT], bf16, isOutput=False)
    dwt = nc.declare_dram_parameter("dwt", [P, NKC * RANK], bf16, isOutput=False)
    upw = nc.declare_dram_parameter("upw", [RANK, D_OUT], bf16, isOutput=False)
    maskt = nc.declare_dram_parameter("maskt", [RANK, NT], f32, isOutput=False)
    out = nc.declare_dram_parameter("out", [NT, D_OUT], bf16, isOutput=True)

    HK = NKC // 2  # ki chunks per ht half-tile

    with tile.TileContext(nc) as tc:
        with (
            tc.tile_pool(name="const", bufs=1) as const,
            tc.tile_pool(name="ht", bufs=6) as ht_pool,
            tc.tile_pool(name="resT", bufs=2) as resT_pool,
            tc.tile_pool(name="outsb", bufs=2) as out_pool,
            tc.tile_pool(name="psum_dn", bufs=2, space="PSUM") as psum_dn_pool,
            tc.tile_pool(name="psum_up", bufs=3, space="PSUM") as psum_up_pool,
        ):
            dwt_sb = const.tile([P, NKC * RANK], bf16)
            upT_sb = const.tile([RANK, D_OUT], bf16)
            maskT_sb = const.tile([RANK, NT], f32)
            # Queue assignment: loads split over the SWDGE (gpsimd) and the
            # scalar HWDGE queue so both 2 MiB halves of a tile stream
            # concurrently and down matmuls never starve; stores own the
            # Sync HWDGE queue exclusively, so a store waiting on copies
            # can never head-of-line-block a load.
            nc.gpsimd.dma_start(out=dwt_sb[:], in_=dwt[:, :])
            nc.scalar.dma_start(out=upT_sb[:], in_=upw[:, :])
            nc.scalar.dma_start(out=maskT_sb[:], in_=maskt[:, :])

            copy_engines = [nc.vector.tensor_copy, nc.scalar.copy]

            for tt in range(NTILES):
                load_engines = [nc.gpsimd, nc.scalar]
                ht_halves = []
                for hh in range(2):
                    ht_sb = ht_pool.tile([P, HK * TT], bf16)
                    load_engines[hh].dma_start(
                        out=ht_sb[:],
                        in_=ht[tt * P:(tt + 1) * P,
                               hh * HK * TT:(hh + 1) * HK * TT],
                    )
                    ht_halves.append(ht_sb)

                psum_dn = psum_dn_pool.tile([RANK, TT], f32)
                for ki in range(NKC):
                    nc.tensor.matmul(
                        psum_dn[:],
                        lhsT=dwt_sb[:, ki * RANK:(ki + 1) * RANK],
                        rhs=ht_halves[ki // HK][:, (ki % HK) * TT:(ki % HK + 1) * TT],
                        start=(ki == 0),
                        stop=(ki == NKC - 1),
                    )

                resT = resT_pool.tile([RANK, TT], bf16)
                nc.vector.tensor_mul(
                    resT[:],
                    maskT_sb[:, tt * TT:(tt + 1) * TT],
                    psum_dn[:],
                )

                for j in range(NJ):
                    out_sb = out_pool.tile([P, D_OUT], bf16)
                    # 2 up-matmuls per 2-bank psum tile -> FD=1024 copies
                    # (PSUM-f32 copies run at 1x; fewer ops amortize the
                    # 120/172-cycle per-op overhead). 2 DVE + 2 ACT per
                    # chunk keeps both engines level.
                    for op in range(NOT // 2):
                        psum_up = psum_up_pool.tile([P, 2 * OT], f32)
                        for oi in range(2):
                            o = op * 2 + oi
                            nc.tensor.matmul(
                                psum_up[:, oi * OT:(oi + 1) * OT],
                                lhsT=resT[:, j * P:(j + 1) * P],
                                rhs=upT_sb[:, o * OT:(o + 1) * OT],
                                start=True,
                                stop=True,
                            )
                        cp = copy_engines[op % 2]
                        cp(
                            out=out_sb[:, op * 2 * OT:(op + 1) * 2 * OT],
                            in_=psum_up[:],
                        )
                    row = (tt * NJ + j) * P
                    nc.sync.dma_start(out=out[row:row + P, :], in_=out_sb[:])

    nc.finalize()
    return nc


def _get_program():
    if "nc" not in _CACHE:
        _CACHE["nc"] = _build_program()
    return _CACHE["nc"]


def prepare_in_maps(hidden_states, down_w, up_w, top_k_values, top_k_indices):
    import ml_dtypes

    bf16 = ml_dtypes.bfloat16

    h = np.asarray(hidden_states, dtype=np.float32)
    dw = np.asarray(down_w, dtype=np.float32)
    uw = np.asarray(up_w, dtype=np.float32)
    vals = np.asarray(top_k_values, dtype=np.float32)
    idx = np.asarray(top_k_indices).astype(np.int64)

    # dwT[p, ki*64 + r] = dw[r, ki*128 + p]
    dwT = np.ascontiguousarray(
        dw.reshape(RANK, NKC, P).transpose(2, 1, 0).reshape(P, NKC * RANK)
    ).astype(bf16)
    upT = np.ascontiguousarray(uw.T).astype(bf16)  # [64, 4096]

    # dense scatter of top-k values: mask[n, r] = val[n, k] where idx[n,k]==r
    mask = np.zeros((N, RANK), dtype=np.float32)
    rows = np.arange(N)[:, None]
    mask[rows, idx] = vals

    in_maps = []
    for c in range(NCORES):
        s = slice(c * NT, (c + 1) * NT)
        # ht[tt*128 + p, ki*512 + u] = h[c*NT + tt*512 + u, ki*128 + p]
        ht = (
            h[s]
            .reshape(NTILES, TT, NKC, P)
            .transpose(0, 3, 2, 1)
            .reshape(NTILES * P, NKC * TT)
            .astype(bf16)
        )
        maskT = np.ascontiguousarray(mask[s].T)  # [64, 2048] f32
        in_maps.append(
            {
                "ht": np.ascontiguousarray(ht),
                "dwt": dwT,
                "upw": upT,
                "maskt": maskT,
            }
        )
    return in_maps


def kernel(hidden_states, down_w, up_w, top_k_values, top_k_indices, **_kw):
    from concourse.bass_utils import run_bass_kernel_spmd

    nc = _get_program()
    in_maps = prepare_in_maps(
        hidden_states, down_w, up_w, top_k_values, top_k_indices
    )
    res = run_bass_kernel_spmd(nc, in_maps, core_ids=list(range(NCORES)))
    return np.concatenate(
        [_unpack_out(r["out"]) for r in res.results], axis=0
    )


def _unpack_out(o):
    return np.asarray(o, dtype=np.float32)


# revision 18
# speedup vs baseline: 1.1506x; 1.1506x over previous
"""MoE LoRA linear layer kernel for Trainium2, data-parallel over 8 NeuronCores.

Math (per token n):
    down = h @ down_w.T                      [N, 64]
    mask[n, r] = val[n, k] if idx[n, k] == r else 0   (indices distinct per row)
    out = (down * mask) @ up_w.T             [N, 4096]

Sharding: tokens split 8 ways (2048/core); LoRA weights replicated.

Key layout decisions (all host-side prep; HW does both matmuls + masking):
  - h is transposed + bf16-cast on the host and packed so each token tile
    is one fully contiguous 4 MiB DMA ([128, 32*512] per tile).  This
    removes the 512 on-device PE transposes + 8.4M elements of PSUM->SBUF
    copies the natural-layout path needs.
  - The top-k scatter (idx/val -> dense [64, NT] maskT) is host-packed like
    the baseline's idx/val chunk repack; the value multiply happens on
    device (DVE) against the down-projection PSUM result.
  - Output is stored bf16 (2e-2 rel-err budget; bf16 ~4e-3) and upcast on
    the host, halving store traffic.

Per-core pipeline (4 token tiles of 512):
  1. one 4 MiB contiguous DMA loads hT tile [128, 32*512] bf16
  2. 32 accumulating bf16 matmuls -> psum_dn [64, 512]
  3. resT = psum_dn * maskT slice (DVE, bf16 out)
  4. per 128-token chunk: 8 matmuls [K=64, N=512] -> psum, DVE/ACT copy
     (bf16 downcast) into out_sb [128, 4096], one 1 MiB store per chunk

HBM traffic per core: 16 MiB in + 16 MiB out + ~1.3 MiB weights ~= 33 MiB,
vs ~50 MiB PE/DVE-bound work in the f32 natural-layout baseline.
"""

import sys

for p in ("/opt/trn_rl_repo", "/opt/pypackages"):
    if p not in sys.path:
        sys.path.insert(0, p)

import numpy as np

N, D_IN, D_OUT, RANK, TOPK = 16384, 4096, 4096, 64, 8
NCORES = 8
NT = N // NCORES          # tokens per core = 2048
P = 128                   # partitions
TT = 512                  # token tile (down-matmul free dim, one PSUM bank)
NKC = D_IN // P           # 32 contraction chunks for down proj
NTILES = NT // TT         # 4 token tiles per core
NJ = TT // P              # 4 x 128-token chunks per tile
OT = 512                  # output col tile (one PSUM bank)
NOT = D_OUT // OT         # 8 output col tiles

_CACHE = {}


def _build_program():
    import concourse.bacc as bacc
    import concourse.mybir as mybir
    from concourse import tile

    f32 = mybir.dt.float32
    bf16 = mybir.dt.bfloat16
    # Bacc (not plain Bass): its finalize() runs move_matmul_waits_to_-
    # ldweights + generate_event_semaphores, which split semaphore waits to
    # satisfy the TRN2 one-wait-per-instruction constraint.
    nc = bacc.Bacc()

    ht = nc.declare_dram_parameter("ht", [NTILES * P, NKC * TT], bf16, isOutput=False)
    dwt = nc.declare_dram_parameter("dwt", [P, NKC * RANK], bf16, isOutput=False)
    upw = nc.declare_dram_parameter("upw", [RANK, D_OUT], bf16, isOutput=False)
    maskt = nc.declare_dram_parameter("maskt", [RANK, NT], f32, isOutput=False)
    out = nc.declare_dram_parameter("out", [NT, D_OUT], bf16, isOutput=True)

    HK = NKC // 2  # ki chunks per ht half-tile

    with tile.TileContext(nc) as tc:
        with (
            tc.tile_pool(name="const", bufs=1) as const,
            tc.tile_pool(name="ht", bufs=6) as ht_pool,
            tc.tile_pool(name="resT", bufs=2) as resT_pool,
            tc.tile_pool(name="outsb", bufs=2) as out_pool,
            tc.tile_pool(name="psum_dn", bufs=2, space="PSUM") as psum_dn_pool,
            tc.tile_pool(name="psum_up", bufs=3, space="PSUM") as psum_up_pool,
        ):
            dwt_sb = const.tile([P, NKC * RANK], bf16)
            upT_sb = const.tile([RANK, D_OUT], bf16)
            maskT_sb = const.tile([RANK, NT], f32)
            # Queue assignment rules learned from traces:
            #  - loads need a FIFO with no compute-dependent waits ahead of
            #    them -> gpsimd (SWDGE) queue carries ONLY loads;
            #  - stores wait on copies, so they get their own queue -> sync
            #    (HWDGE, the faster ring, good for the 16.8 MB store stream);
            #  - the scalar engine's FIFO is poisoned by its PSUM copies ->
            #    never issue DMA from it.
            nc.gpsimd.dma_start(out=dwt_sb[:], in_=dwt[:, :])

            copy_engines = [nc.vector.tensor_copy, nc.scalar.copy]

            for tt in range(NTILES):
                ht_halves = []
                for hh in range(2):
                    ht_sb = ht_pool.tile([P, HK * TT], bf16)
                    nc.gpsimd.dma_start(
                        out=ht_sb[:],
                        in_=ht[tt * P:(tt + 1) * P,
                               hh * HK * TT:(hh + 1) * HK * TT],
                    )
                    ht_halves.append(ht_sb)
                if tt == 0:
                    # needed only from the first up-proj / mask multiply on
                    nc.gpsimd.dma_start(out=upT_sb[:], in_=upw[:, :])
                    nc.gpsimd.dma_start(out=maskT_sb[:], in_=maskt[:, :])

                psum_dn = psum_dn_pool.tile([RANK, TT], f32)
                for ki in range(NKC):
                    nc.tensor.matmul(
                        psum_dn[:],
                        lhsT=dwt_sb[:, ki * RANK:(ki + 1) * RANK],
                        rhs=ht_halves[ki // HK][:, (ki % HK) * TT:(ki % HK + 1) * TT],
                        start=(ki == 0),
                        stop=(ki == NKC - 1),
                    )

                resT = resT_pool.tile([RANK, TT], bf16)
                nc.vector.tensor_mul(
                    resT[:],
                    maskT_sb[:, tt * TT:(tt + 1) * TT],
                    psum_dn[:],
                )

                for j in range(NJ):
                    out_sb = out_pool.tile([P, D_OUT], bf16)
                    # 2 up-matmuls per 2-bank psum tile -> FD=1024 copies
                    # (PSUM-f32 copies run at 1x; fewer ops amortize the
                    # 120/172-cycle per-op overhead). 2 DVE + 2 ACT per
                    # chunk keeps both engines level.
                    for op in range(NOT // 2):
                        psum_up = psum_up_pool.tile([P, 2 * OT], f32)
                        for oi in range(2):
                            o = op * 2 + oi
                            nc.tensor.matmul(
                                psum_up[:, oi * OT:(oi + 1) * OT],
                                lhsT=resT[:, j * P:(j + 1) * P],
                                rhs=upT_sb[:, o * OT:(o + 1) * OT],
                                start=True,
                                stop=True,
                            )
                        cp = copy_engines[op % 2]
                        cp(
                            out=out_sb[:, op * 2 * OT:(op + 1) * 2 * OT],
                            in_=psum_up[:],
                        )
                    row = (tt * NJ + j) * P
                    nc.sync.dma_start(out=out[row:row + P, :], in_=out_sb[:])

    nc.finalize()
    return nc


def _get_program():
    if "nc" not in _CACHE:
        _CACHE["nc"] = _build_program()
    return _CACHE["nc"]


def prepare_in_maps(hidden_states, down_w, up_w, top_k_values, top_k_indices):
    import ml_dtypes

    bf16 = ml_dtypes.bfloat16

    h = np.asarray(hidden_states, dtype=np.float32)
    dw = np.asarray(down_w, dtype=np.float32)
    uw = np.asarray(up_w, dtype=np.float32)
    vals = np.asarray(top_k_values, dtype=np.float32)
    idx = np.asarray(top_k_indices).astype(np.int64)

    # dwT[p, ki*64 + r] = dw[r, ki*128 + p]
    dwT = np.ascontiguousarray(
        dw.reshape(RANK, NKC, P).transpose(2, 1, 0).reshape(P, NKC * RANK)
    ).astype(bf16)
    upT = np.ascontiguousarray(uw.T).astype(bf16)  # [64, 4096]

    # dense scatter of top-k values: mask[n, r] = val[n, k] where idx[n,k]==r
    mask = np.zeros((N, RANK), dtype=np.float32)
    rows = np.arange(N)[:, None]
    mask[rows, idx] = vals

    in_maps = []
    for c in range(NCORES):
        s = slice(c * NT, (c + 1) * NT)
        # ht[tt*128 + p, ki*512 + u] = h[c*NT + tt*512 + u, ki*128 + p]
        ht = (
            h[s]
            .reshape(NTILES, TT, NKC, P)
            .transpose(0, 3, 2, 1)
            .reshape(NTILES * P, NKC * TT)
            .astype(bf16)
        )
        maskT = np.ascontiguousarray(mask[s].T)  # [64, 2048] f32
        in_maps.append(
            {
                "ht": np.ascontiguousarray(ht),
                "dwt": dwT,
                "upw": upT,
                "maskt": maskT,
            }
        )
    return in_maps


def kernel(hidden_states, down_w, up_w, top_k_values, top_k_indices, **_kw):
    from concourse.bass_utils import run_bass_kernel_spmd

    nc = _get_program()
    in_maps = prepare_in_maps(
        hidden_states, down_w, up_w, top_k_values, top_k_indices
    )
    res = run_bass_kernel_spmd(nc, in_maps, core_ids=list(range(NCORES)))
    return np.concatenate(
        [_unpack_out(r["out"]) for r in res.results], axis=0
    )


def _unpack_out(o):
    return np.asarray(o, dtype=np.float32)


# revision 20
# speedup vs baseline: 1.1885x; 1.0330x over previous
"""MoE LoRA linear layer kernel for Trainium2, data-parallel over 8 NeuronCores.

Math (per token n):
    down = h @ down_w.T                      [N, 64]
    mask[n, r] = val[n, k] if idx[n, k] == r else 0   (indices distinct per row)
    out = (down * mask) @ up_w.T             [N, 4096]

Sharding: tokens split 8 ways (2048/core); LoRA weights replicated.

Key layout decisions (all host-side prep; HW does both matmuls + masking):
  - h is transposed + bf16-cast on the host and packed so each token tile
    is one fully contiguous 4 MiB DMA ([128, 32*512] per tile).  This
    removes the 512 on-device PE transposes + 8.4M elements of PSUM->SBUF
    copies the natural-layout path needs.
  - The top-k scatter (idx/val -> dense [64, NT] maskT) is host-packed like
    the baseline's idx/val chunk repack; the value multiply happens on
    device (DVE) against the down-projection PSUM result.
  - Output is stored bf16 (2e-2 rel-err budget; bf16 ~4e-3) and upcast on
    the host, halving store traffic.

Per-core pipeline (4 token tiles of 512):
  1. one 4 MiB contiguous DMA loads hT tile [128, 32*512] bf16
  2. 32 accumulating bf16 matmuls -> psum_dn [64, 512]
  3. resT = psum_dn * maskT slice (DVE, bf16 out)
  4. per 128-token chunk: 8 matmuls [K=64, N=512] -> psum, DVE/ACT copy
     (bf16 downcast) into out_sb [128, 4096], one 1 MiB store per chunk

HBM traffic per core: 16 MiB in + 16 MiB out + ~1.3 MiB weights ~= 33 MiB,
vs ~50 MiB PE/DVE-bound work in the f32 natural-layout baseline.
"""

import sys

for p in ("/opt/trn_rl_repo", "/opt/pypackages"):
    if p not in sys.path:
        sys.path.insert(0, p)

import numpy as np

N, D_IN, D_OUT, RANK, TOPK = 16384, 4096, 4096, 64, 8
NCORES = 8
NT = N // NCORES          # tokens per core = 2048
P = 128                   # partitions
TT = 512                  # token tile (down-matmul free dim, one PSUM bank)
NKC = D_IN // P           # 32 contraction chunks for down proj
NTILES = NT // TT         # 4 token tiles per core
NJ = TT // P              # 4 x 128-token chunks per tile
OT = 512                  # output col tile (one PSUM bank)
NOT = D_OUT // OT         # 8 output col tiles

_CACHE = {}


def _build_program():
    import concourse.bacc as bacc
    import concourse.mybir as mybir
    from concourse import tile

    f32 = mybir.dt.float32
    bf16 = mybir.dt.bfloat16
    # Bacc (not plain Bass): its finalize() runs move_matmul_waits_to_-
    # ldweights + generate_event_semaphores, which split semaphore waits to
    # satisfy the TRN2 one-wait-per-instruction constraint.
    nc = bacc.Bacc()

    ht = nc.declare_dram_parameter("ht", [NTILES * P, NKC * TT], bf16, isOutput=False)
    dwt = nc.declare_dram_parameter("dwt", [P, NKC * RANK], bf16, isOutput=False)
    upw = nc.declare_dram_parameter("upw", [RANK, D_OUT], bf16, isOutput=False)
    maskt = nc.declare_dram_parameter("maskt", [RANK, NT], f32, isOutput=False)
    out = nc.declare_dram_parameter("out", [NT, D_OUT], bf16, isOutput=True)

    HK = NKC // 2  # ki chunks per ht half-tile

    with tile.TileContext(nc) as tc:
        with (
            tc.tile_pool(name="const", bufs=1) as const,
            tc.tile_pool(name="ht", bufs=6) as ht_pool,
            tc.tile_pool(name="resT", bufs=2) as resT_pool,
            tc.tile_pool(name="outsb", bufs=2) as out_pool,
            tc.tile_pool(name="psum_dn", bufs=2, space="PSUM") as psum_dn_pool,
            tc.tile_pool(name="psum_up", bufs=3, space="PSUM") as psum_up_pool,
        ):
            dwt_sb = const.tile([P, NKC * RANK], bf16)
            upT_sb = const.tile([RANK, D_OUT], bf16)
            maskT_sb = const.tile([RANK, NT], f32)
            # Queue assignment rules learned from traces:
            #  - loads go on the sync HWDGE queue, which carries ONLY loads
            #    (a FIFO with compute-dependent waits ahead of a load starves
            #    the PE);
            #  - stores wait on copies, so they get their own queue; SWDGE
            #    (gpsimd) stores overlap HWDGE loads, whereas HWDGE stores
            #    get starved behind a busy SWDGE load stream (measured);
            #  - the scalar engine's FIFO is poisoned by its PSUM copies ->
            #    never issue DMA from it.
            nc.sync.dma_start(out=dwt_sb[:], in_=dwt[:, :])
            # upw/maskt ride the store (SWDGE) queue, which is empty until
            # the first up-proj finishes.
            nc.gpsimd.dma_start(out=upT_sb[:], in_=upw[:, :])
            nc.gpsimd.dma_start(out=maskT_sb[:], in_=maskt[:, :])

            copy_engines = [nc.vector.tensor_copy, nc.scalar.copy]

            for tt in range(NTILES):
                ht_halves = []
                for hh in range(2):
                    ht_sb = ht_pool.tile([P, HK * TT], bf16)
                    nc.sync.dma_start(
                        out=ht_sb[:],
                        in_=ht[tt * P:(tt + 1) * P,
                               hh * HK * TT:(hh + 1) * HK * TT],
                    )
                    ht_halves.append(ht_sb)

                psum_dn = psum_dn_pool.tile([RANK, TT], f32)
                for ki in range(NKC):
                    nc.tensor.matmul(
                        psum_dn[:],
                        lhsT=dwt_sb[:, ki * RANK:(ki + 1) * RANK],
                        rhs=ht_halves[ki // HK][:, (ki % HK) * TT:(ki % HK + 1) * TT],
                        start=(ki == 0),
                        stop=(ki == NKC - 1),
                    )

                resT = resT_pool.tile([RANK, TT], bf16)
                nc.vector.tensor_mul(
                    resT[:],
                    maskT_sb[:, tt * TT:(tt + 1) * TT],
                    psum_dn[:],
                )

                for j in range(NJ):
                    out_sb = out_pool.tile([P, D_OUT], bf16)
                    # 2 up-matmuls per 2-bank psum tile -> FD=1024 copies
                    # (PSUM-f32 copies run at 1x; fewer ops amortize the
                    # 120/172-cycle per-op overhead). 2 DVE + 2 ACT per
                    # chunk keeps both engines level.
                    for op in range(NOT // 2):
                        psum_up = psum_up_pool.tile([P, 2 * OT], f32)
                        for oi in range(2):
                            o = op * 2 + oi
                            nc.tensor.matmul(
                                psum_up[:, oi * OT:(oi + 1) * OT],
                                lhsT=resT[:, j * P:(j + 1) * P],
                                rhs=upT_sb[:, o * OT:(o + 1) * OT],
                                start=True,
                                stop=True,
                            )
                        cp = copy_engines[op % 2]
                        cp(
                            out=out_sb[:, op * 2 * OT:(op + 1) * 2 * OT],
                            in_=psum_up[:],
                        )
                    row = (tt * NJ + j) * P
                    nc.gpsimd.dma_start(out=out[row:row + P, :], in_=out_sb[:])

    nc.finalize()
    return nc


def _get_program():
    if "nc" not in _CACHE:
        _CACHE["nc"] = _build_program()
    return _CACHE["nc"]


def prepare_in_maps(hidden_states, down_w, up_w, top_k_values, top_k_indices):
    import ml_dtypes

    bf16 = ml_dtypes.bfloat16

    h = np.asarray(hidden_states, dtype=np.float32)
    dw = np.asarray(down_w, dtype=np.float32)
    uw = np.asarray(up_w, dtype=np.float32)
    vals = np.asarray(top_k_values, dtype=np.float32)
    idx = np.asarray(top_k_indices).astype(np.int64)

    # dwT[p, ki*64 + r] = dw[r, ki*128 + p]
    dwT = np.ascontiguousarray(
        dw.reshape(RANK, NKC, P).transpose(2, 1, 0).reshape(P, NKC * RANK)
    ).astype(bf16)
    upT = np.ascontiguousarray(uw.T).astype(bf16)  # [64, 4096]

    # dense scatter of top-k values: mask[n, r] = val[n, k] where idx[n,k]==r
    mask = np.zeros((N, RANK), dtype=np.float32)
    rows = np.arange(N)[:, None]
    mask[rows, idx] = vals

    in_maps = []
    for c in range(NCORES):
        s = slice(c * NT, (c + 1) * NT)
        # ht[tt*128 + p, ki*512 + u] = h[c*NT + tt*512 + u, ki*128 + p]
        ht = (
            h[s]
            .reshape(NTILES, TT, NKC, P)
            .transpose(0, 3, 2, 1)
            .reshape(NTILES * P, NKC * TT)
            .astype(bf16)
        )
        maskT = np.ascontiguousarray(mask[s].T)  # [64, 2048] f32
        in_maps.append(
            {
                "ht": np.ascontiguousarray(ht),
                "dwt": dwT,
                "upw": upT,
                "maskt": maskT,
            }
        )
    return in_maps


def kernel(hidden_states, down_w, up_w, top_k_values, top_k_indices, **_kw):
    from concourse.bass_utils import run_bass_kernel_spmd

    nc = _get_program()
    in_maps = prepare_in_maps(
        hidden_states, down_w, up_w, top_k_values, top_k_indices
    )
    res = run_bass_kernel_spmd(nc, in_maps, core_ids=list(range(NCORES)))
    return np.concatenate(
        [_unpack_out(r["out"]) for r in res.results], axis=0
    )


def _unpack_out(o):
    return np.asarray(o, dtype=np.float32)
